# revision 1
# baseline (speedup 1.0000x reference)
"""Trainium2 Bass kernel for nn_Attention_62362925138174.

Reference computation (per batch b, with xf = x[b].reshape(C, N), N = H*W):
    q = Wq @ xf            [8,  N]
    k = Wk @ xf            [8,  N]
    v = Wv @ xf            [C,  N]
    score[n, m] = q[:, n] . k[:, m]
    P = softmax(score, axis=n)          (per-column softmax)
    out[c, m] = sum_n v[c, n] P[n, m]
    att = gamma * out + xf

Kernel strategy (8 cores = 4 batches x 2 column-halves of N):
  - Fold the q/k projections: score = xf^T (Wq^T Wk) xf.  On device we
    compute GT = Wk^T Wq once, then kg = GT^T @ xk for the core's half of
    the columns, so score tiles are matmul(lhsT=xf[:, ntile], rhs=kg).
  - Softmax without max-subtraction (scores are O(+-5), exp is safe in f32);
    the normalizer is obtained by appending a ones-row to V^T so one PSUM
    accumulation chain yields both V @ E and colsum(E).
  - 1/colsum is computed as exp(-ln(colsum) + ln(gamma)) on ScalarE: the
    ln/exp/copy functions share one activation table set, and this also
    folds the gamma scale in for free (DVE's reciprocal is ~6.5 ns/elem).
  - All matmuls run in bf16 (1 cycle/column on PE) with f32 PSUM
    accumulation; the residual add uses the exact f32 input, which dominates
    the output, so overall error stays ~1e-4.
"""

import numpy as np

import concourse.bass as bass
import concourse.bacc as bacc
import concourse.tile as tile
from concourse import mybir
from concourse.bass_utils import run_bass_kernel_spmd

# Problem shape (hardcoded per contract).
B, C, H, W = 4, 64, 64, 64
N = H * W           # 4096
MHALF = N // 2      # 2048 columns of the score/output handled per core
NT = N // 128       # 32 row-tiles of the score matrix
N_CORES = 8

F32 = mybir.dt.float32
BF16 = mybir.dt.bfloat16
_NP_BF16 = mybir.dt.np(BF16)

_PROGRAM = None


def _build_program() -> bass.Bass:
    nc = bacc.Bacc()

    xfp_d = nc.declare_dram_parameter("xfp", [128, N], BF16, isOutput=False)
    xkp_d = nc.declare_dram_parameter("xkp", [128, MHALF], BF16, isOutput=False)
    xkf_d = nc.declare_dram_parameter("xkf", [C, MHALF], F32, isOutput=False)
    gt_d = nc.declare_dram_parameter("gt", [128, C], BF16, isOutput=False)
    wvT_d = nc.declare_dram_parameter("wvT", [128, C], BF16, isOutput=False)
    g_d = nc.declare_dram_parameter("g", [1, 1], F32, isOutput=False)
    out_d = nc.declare_dram_parameter("out", [C, MHALF], F32, isOutput=True)

    EXP = mybir.ActivationFunctionType.Exp
    LN = mybir.ActivationFunctionType.Ln

    # Pin the activation table set to natural_log_exp_and_others (covers
    # Exp, Ln and Copy) so the whole kernel needs exactly one table load.
    from concourse.hw_specs import get_activation_tables

    act_sets = list(get_activation_tables(nc.m.arch))
    nle_id = act_sets.index("natural_log_exp_and_others")

    with TileCtx(nc) as (tc, sing, epool, apool, psS, psO):
        nc.scalar.add_instruction(
            mybir.InstLoadActFuncSet(
                name=nc.get_next_instruction_name(),
                act_func_set_id=nle_id,
                ins=[],
                outs=[],
            )
        )
        # ---- input loads, spread over engine DGE queues ----
        g_sb = sing.tile([1, 1], F32, name="g_sb")
        nc.sync.dma_start(out=g_sb, in_=g_d[:, :])
        gt_sb = sing.tile([128, C], BF16, name="gt_sb")
        nc.sync.dma_start(out=gt_sb, in_=gt_d[:, :])
        xkp_sb = sing.tile([128, MHALF], BF16, name="xkp_sb")
        for i in range(2):
            nc.sync.dma_start(
                out=xkp_sb[:, i * 1024 : (i + 1) * 1024],
                in_=xkp_d[:, i * 1024 : (i + 1) * 1024],
            )
        wvT_sb = sing.tile([128, C], BF16, name="wvT_sb")
        nc.sync.dma_start(out=wvT_sb, in_=wvT_d[:, :])
        xfp_sb = sing.tile([128, N], BF16, name="xfp_sb")
        for i in range(4):
            nc.scalar.dma_start(
                out=xfp_sb[:, i * 1024 : (i + 1) * 1024],
                in_=xfp_d[:, i * 1024 : (i + 1) * 1024],
            )
        xkf_sb = sing.tile([C, MHALF], F32, name="xkf_sb")
        for i in range(2):
            nc.sync.dma_start(
                out=xkf_sb[:, i * 1024 : (i + 1) * 1024],
                in_=xkf_d[:, i * 1024 : (i + 1) * 1024],
            )

        # ---- kg = G @ xk ([C, MHALF]); G^T supplied pre-padded by host ----
        kg_sb = sing.tile([128, MHALF], BF16, name="kg_sb")
        nc.vector.memset(kg_sb[C:128, :], 0.0)
        for hh in range(2):
            kgp = psS.tile([128, 1024], F32, tag="S", name="kgp")
            for cc in range(2):
                lo = hh * 1024 + cc * 512
                nc.tensor.matmul(
                    kgp[0:C, cc * 512 : (cc + 1) * 512],
                    lhsT=gt_sb,
                    rhs=xkp_sb[:, lo : lo + 512],
                    start=True,
                    stop=True,
                )
                # per-512 copies, split over ScalarE/VectorE so the first
                # score matmul can start as soon as its kg chunk is ready
                if cc == 0:
                    nc.scalar.copy(
                        out=kg_sb[0:C, lo : lo + 512],
                        in_=kgp[0:C, cc * 512 : (cc + 1) * 512],
                    )
                else:
                    nc.vector.tensor_copy(
                        out=kg_sb[0:C, lo : lo + 512],
                        in_=kgp[0:C, cc * 512 : (cc + 1) * 512],
                    )

        # ---- vaugT[n, 0:64] = (Wv @ xf)^T tile, vaugT[n, 64] = 1 ----
        # Chunks of 8 tiles interleaved into early iterations (chunk c is
        # needed only from iteration t = 8c) so the exp loop starts early.
        vaug_sb = sing.tile([128, NT * 65], BF16, name="vaug_sb")
        vaug3 = vaug_sb.rearrange("p (t u) -> p t u", u=65)
        nc.vector.memset(vaug3[:, :, 64:65], 1.0)

        def emit_vt_chunk(vv):
            vtp = psS.tile([128, 1024], F32, tag="S", name="vtp")
            for i in range(8):
                t = vv * 8 + i
                nc.tensor.matmul(
                    vtp[:, i * 64 : (i + 1) * 64],
                    lhsT=xfp_sb[:, t * 128 : (t + 1) * 128],
                    rhs=wvT_sb,
                    start=True,
                    stop=True,
                )
            nc.vector.tensor_copy(
                out=vaug3[:, vv * 8 : (vv + 1) * 8, 0:64],
                in_=vtp[:, 0:512].rearrange("p (i u) -> p i u", u=64),
            )

        # ln(gamma): needed only in the tail; emitted after the kg copies so
        # it never blocks the scalar queue during the prologue.
        ln_g = sing.tile([1, 1], F32, name="ln_g")
        nc.scalar.activation(out=ln_g, in_=g_sb, func=LN)

        # ---- main loop: score -> exp -> accumulate V_aug @ E ----
        O_ps = psO.tile([65, MHALF], F32, name="O_ps")
        for t in range(NT):
            lhsT_t = xfp_sb[:, t * 128 : (t + 1) * 128]
            Es = []
            for h in range(2):
                S = psS.tile([128, 1024], F32, tag="S", name="S_ps")
                for cc in range(2):
                    lo = h * 1024 + cc * 512
                    nc.tensor.matmul(
                        S[:, cc * 512 : (cc + 1) * 512],
                        lhsT=lhsT_t,
                        rhs=kg_sb[:, lo : lo + 512],
                        start=True,
                        stop=True,
                    )
                E = epool.tile([128, 1024], BF16, tag="E", name="E_sb")
                nc.scalar.activation(out=E, in_=S, func=EXP)
                Es.append(E)
            if t in (0, 2, 4, 6):
                emit_vt_chunk(t // 2)
            va_t = vaug3[:, t, :]
            for h in range(2):
                for cc in range(2):
                    lo = h * 1024 + cc * 512
                    nc.tensor.matmul(
                        O_ps[:, lo : lo + 512],
                        lhsT=va_t,
                        rhs=Es[h][:, cc * 512 : (cc + 1) * 512],
                        start=(t == 0),
                        stop=(t == NT - 1),
                    )

        # ---- normalize + gamma + residual, store ----
        # rcp[m] = gamma / colsum[m] = exp(-ln(colsum[m]) + ln(gamma)).
        # All ScalarE work first (Tile serializes same-PSUM readers in
        # emission order), then GpSimd broadcasts, then DVE mul/add chains.
        bcss = []
        for half in range(2):
            hsl = slice(half * 1024, (half + 1) * 1024)
            lnt = apool.tile([1, 1024], F32, tag="lnt", name="lnt")
            nc.scalar.activation(out=lnt, in_=O_ps[64:65, hsl], func=LN)
            rcp = apool.tile([1, 1024], BF16, tag="rcp", name="rcp")
            nc.scalar.activation(
                out=rcp, in_=lnt, func=EXP, scale=-1.0, bias=ln_g[0:1, 0:1]
            )
            bcs = apool.tile([C, 1024], BF16, tag="bcs", name="bcs")
            nc.gpsimd.partition_broadcast(bcs, rcp)
            bcss.append(bcs)
        for j in range(4):
            sl = slice(j * 512, (j + 1) * 512)
            bsl = slice((j % 2) * 512, (j % 2) * 512 + 512)
            tmp = apool.tile([C, 512], F32, tag="tmp", name="tmp")
            nc.vector.tensor_mul(tmp, O_ps[0:C, sl], bcss[j // 2][:, bsl])
            att = apool.tile([C, 512], F32, tag="att", name="att")
            nc.vector.tensor_add(att, tmp, xkf_sb[:, sl])
            nc.sync.dma_start(out=out_d[:, sl], in_=att)

    nc.finalize()
    return nc


class TileCtx:
    """TileContext plus the tile pools used by the kernel."""

    def __init__(self, nc: bass.Bass):
        self.nc = nc

    def __enter__(self):
        from contextlib import ExitStack

        self._stack = ExitStack()
        tc = self._stack.enter_context(tile.TileContext(self.nc))
        sing = self._stack.enter_context(tc.tile_pool(name="sing", bufs=1))
        epool = self._stack.enter_context(tc.tile_pool(name="epool", bufs=6))
        apool = self._stack.enter_context(tc.tile_pool(name="apool", bufs=4))
        psS = self._stack.enter_context(tc.tile_pool(name="psS", bufs=2, space="PSUM"))
        psO = self._stack.enter_context(tc.tile_pool(name="psO", bufs=1, space="PSUM"))
        return tc, sing, epool, apool, psS, psO

    def __exit__(self, *exc):
        return self._stack.__exit__(*exc)


def get_program() -> bass.Bass:
    global _PROGRAM
    if _PROGRAM is None:
        _PROGRAM = _build_program()
    return _PROGRAM


def make_in_maps(x, Wq, Wk, Wv, gamma):
    """Shard the full inputs into per-core input maps (host-side prep only:
    reshape/slice, zero-pad the contraction dim to 128, cast to bf16)."""
    x = np.ascontiguousarray(np.asarray(x, dtype=np.float32))
    Wq = np.asarray(Wq, dtype=np.float32)
    Wk = np.asarray(Wk, dtype=np.float32)
    Wv = np.asarray(Wv, dtype=np.float32)
    gamma = np.asarray(gamma, dtype=np.float32).reshape(1, 1)

    def padk(a):  # [k, m] -> [128, m] zero-padded, bf16
        out = np.zeros((128, a.shape[1]), dtype=_NP_BF16)
        out[: a.shape[0]] = a.astype(_NP_BF16)
        return out

    # G^T = Wk^T Wq: pure weight preprocessing (score = xf^T (Wq^T Wk) xf)
    gt_p = padk(Wk.T @ Wq)   # [128, 64]
    wvT_p = padk(Wv.T)       # [128, 64]

    in_maps = []
    for core in range(N_CORES):
        b, h = divmod(core, 2)
        xf = x[b].reshape(C, N)
        xk = xf[:, h * MHALF : (h + 1) * MHALF]
        in_maps.append(
            {
                "xfp": padk(xf),
                "xkp": padk(xk),
                "xkf": np.ascontiguousarray(xk),
                "gt": gt_p,
                "wvT": wvT_p,
                "g": gamma,
            }
        )
    return in_maps


def gather(results):
    out = np.empty((B, C, N), dtype=np.float32)
    for core in range(N_CORES):
        b, h = divmod(core, 2)
        out[b][:, h * MHALF : (h + 1) * MHALF] = results[core]["out"]
    return out.reshape(B, C, H, W)


def run(inputs, **spmd_kwargs):
    nc = get_program()
    in_maps = make_in_maps(
        inputs["x"], inputs["Wq"], inputs["Wk"], inputs["Wv"], inputs["gamma"]
    )
    res = run_bass_kernel_spmd(nc, in_maps, core_ids=list(range(N_CORES)), **spmd_kwargs)
    return gather(res.results), res


def kernel(x, Wq, Wk, Wv, gamma):
    out, _ = run({"x": x, "Wq": Wq, "Wk": Wk, "Wv": Wv, "gamma": gamma})
    return out

